# revision 5
# baseline (speedup 1.0000x reference)
# Triplet-margin loss kernel for Trainium2 (Bass/Tile), batch-sharded
# across 8 NeuronCores.
#
# reference math (torch F.pairwise_distance semantics):
#   d_ap[b,p] = || anc[b] - pos[b,p] + eps ||_2
#   d_an[b,n] = || anc[b] - neg[b,n] + eps ||_2
#   loss = mean_{b,p,n} max(d_ap[b,p] - d_an[b,n] + margin, 0)
#
# (eps = 1e-6 shifts d^2 by ~3e-8 relative; it is dropped here, far
# below the fp32 noise floor of the d^2 cancellation itself.)
#
# Slice paths, chosen to balance DVE/ACT/Pool busy time under the DMA
# roofline (~63us for the 25 MiB each core reads):
#   pos slices 0-7:  u = x - anc on GpSimd (plain-AP [128,2048] ops
#     against a DMA-materialized anc2; stride-0 broadcast APs on Pool
#     hammer the SBUF port it shares with DVE and slow DVE ~8x, so
#     they are avoided), then d^2 = sum u^2 reduced on DVE (slices
#     0-5, self-multiply STT) or ACT (slices 6-7, Square w/ accum).
#   neg slices 8-23: dotm2 = -2 sum x*anc via one DVE STT (scalar -2.0
#     folded into op0) + nrm = sum x^2 via one ACT Square w/ accum;
#     d = sqrt((nrm + dotm2) + ||anc||^2) via TT add + biased Sqrt.
# The (p,n) pair combination is two ops on [128, P*N]: a broadcast
# (d_ap + margin) - d_an STT on DVE and a Relu w/ accum on ACT, giving
# the per-partition loss sum per tile. Each core returns [128, NT]
# partial sums; the host sums and scales.

import numpy as np

import concourse.bacc as bacc
import concourse.mybir as mybir
import concourse.tile as tile
from concourse import bass_utils

B, Z = 2048, 1024
NUM_POS, NUM_NEG = 8, 16
NJ = NUM_POS + NUM_NEG
MARGIN = 1.0
N_CORES = 8
BL = B // N_CORES  # 256 rows of anc per core
P = 128
NT = BL // P  # 2 batch-tiles per core

# chunk layout per tile: slice counts per DMA; small final chunks keep
# the post-DMA tail short.
CHUNK_SLICES = [4, 4, 4, 4, 4, 2, 2]
NPOOL = 8  # slices 0..NPOOL-1 take the GpSimd u-path (must cover whole chunks)
POOL_OP_SLICES = 2  # slices per GpSimd subtract op
DVE_U_NRM = 6  # u-slices 0..DVE_U_NRM-1 reduce on DVE, the rest on ACT

F32 = mybir.dt.float32
AF = mybir.ActivationFunctionType
OP = mybir.AluOpType


def _emit(tc, nc, anc, pos, neg, out):
    v = nc.vector
    act = nc.scalar
    gp = nc.gpsimd
    pos2 = pos.rearrange("(b j) z -> b (j z)", j=NUM_POS)  # [BL, 8*Z]
    neg2 = neg.rearrange("(b j) z -> b (j z)", j=NUM_NEG)  # [BL, 16*Z]

    starts = np.cumsum([0] + CHUNK_SLICES).tolist()
    assert starts[-1] == NJ

    def chunk_src(ci, b0):
        j0, j1 = starts[ci], starts[ci + 1]
        if j1 <= NUM_POS:
            return pos2[b0 : b0 + P, j0 * Z : j1 * Z]
        assert j0 >= NUM_POS
        return neg2[b0 : b0 + P, (j0 - NUM_POS) * Z : (j1 - NUM_POS) * Z]

    PW = POOL_OP_SLICES * Z

    with (
        tc.tile_pool(name="xp", bufs=8) as xp,
        tc.tile_pool(name="up", bufs=4) as up,
        tc.tile_pool(name="apool", bufs=2) as apool,
        tc.tile_pool(name="scp", bufs=1) as scp,
        tc.tile_pool(name="smp", bufs=2) as smp,
        tc.tile_pool(name="opool", bufs=1) as opool,
    ):
        osb = opool.tile([P, NT], F32, name="osb")
        dve_scr = scp.tile([P, Z], F32, name="dve_scr")
        act_scr = scp.tile([P, Z], F32, name="act_scr")
        pair = scp.tile([P, NUM_POS * NUM_NEG], F32, name="pair")
        pair_scr = scp.tile([P, NUM_POS * NUM_NEG], F32, name="pair_scr")
        for t in range(NT):
            b0 = t * P
            anc2 = apool.tile([P, PW], F32, name="anc2")
            a_nrm = smp.tile([P, 1], F32, name="a_nrm")
            dotm2 = smp.tile([P, NJ - NPOOL], F32, name="dotm2")
            nrm = smp.tile([P, NJ], F32, name="nrm")
            d2c = smp.tile([P, NJ - NPOOL], F32, name="d2c")
            dt_ = smp.tile([P, NJ], F32, name="dt_")

            # anc2 = [anc | anc]: one HBM load + SBUF->SBUF doubling
            nc.sync.dma_start(anc2[:, 0:Z], anc[b0 : b0 + P, :])
            for rep in range(1, POOL_OP_SLICES):
                nc.sync.dma_start(anc2[:, rep * Z : (rep + 1) * Z], anc2[:, 0:Z])
            anc_in = anc2[:, 0:Z]
            act.activation(
                act_scr[:, :], anc_in, AF.Square, accum_out=a_nrm[:, 0:1]
            )

            chunks = []
            for ci in range(len(CHUNK_SLICES)):
                xt = xp.tile([P, CHUNK_SLICES[ci] * Z], F32, name="xt")
                nc.sync.dma_start(xt[:, :], chunk_src(ci, b0))
                chunks.append(xt)

            def xs_of(jj):
                ci = next(i for i in range(len(starts) - 1) if starts[i + 1] > jj)
                q = jj - starts[ci]
                return chunks[ci][:, q * Z : (q + 1) * Z]

            # GpSimd: u = x - anc for pool slices, POOL_OP_SLICES at a time
            un = {}
            for j0 in range(0, NPOOL, POOL_OP_SLICES):
                ci = next(i for i in range(len(starts) - 1) if starts[i + 1] > j0)
                q = j0 - starts[ci]
                ut = up.tile([P, PW], F32, name="ut")
                gp.tensor_tensor(
                    out=ut[:, :],
                    in0=chunks[ci][:, q * Z : q * Z + PW],
                    in1=anc2[:, :],
                    op=OP.subtract,
                )
                for k in range(POOL_OP_SLICES):
                    un[j0 + k] = ut[:, k * Z : (k + 1) * Z]

            # DVE: d^2 = sum u^2 for its u-slices, then dotm2 for dot slices
            for jj in range(DVE_U_NRM):
                v.scalar_tensor_tensor(
                    out=dve_scr[:, :],
                    in0=un[jj],
                    scalar=1.0,
                    in1=un[jj],
                    op0=OP.bypass,
                    op1=OP.mult,
                    accum_out=nrm[:, jj : jj + 1],
                )
            for jj in range(NPOOL, NJ):
                v.scalar_tensor_tensor(
                    out=dve_scr[:, :],
                    in0=xs_of(jj),
                    scalar=-2.0,
                    in1=anc_in,
                    op0=OP.mult,
                    op1=OP.mult,
                    accum_out=dotm2[:, jj - NPOOL : jj - NPOOL + 1],
                )

            # ACT: remaining u-norms, then x-norms for dot slices
            for jj in range(DVE_U_NRM, NPOOL):
                act.activation(
                    act_scr[:, :], un[jj], AF.Square, accum_out=nrm[:, jj : jj + 1]
                )
            for jj in range(NPOOL, NJ):
                act.activation(
                    act_scr[:, :], xs_of(jj), AF.Square, accum_out=nrm[:, jj : jj + 1]
                )

            # dot slices: d = sqrt((nrm + dotm2) + ||anc||^2)
            v.tensor_tensor(
                out=d2c[:, :], in0=dotm2[:, :], in1=nrm[:, NPOOL:NJ], op=OP.add
            )
            act.activation(
                dt_[:, NPOOL:NJ], d2c[:, :], AF.Sqrt, bias=a_nrm[:, 0:1], scale=1.0
            )
            # pool slices already hold d^2 in nrm
            act.activation(dt_[:, 0:NPOOL], nrm[:, 0:NPOOL], AF.Sqrt)
            # pair[p,n] = (d_ap_p + margin) - d_an_n ; loss sum = sum relu
            v.scalar_tensor_tensor(
                out=pair[:, :].rearrange("p (a b) -> p a b", a=NUM_POS),
                in0=dt_[:, 0:NUM_POS, None].broadcast_to([P, NUM_POS, NUM_NEG]),
                scalar=MARGIN,
                in1=dt_[:, None, NUM_POS:NJ].broadcast_to([P, NUM_POS, NUM_NEG]),
                op0=OP.add,
                op1=OP.subtract,
            )
            act.activation(
                pair_scr[:, :], pair[:, :], AF.Relu, accum_out=osb[:, t : t + 1]
            )
        nc.sync.dma_start(out[:, :], osb[:, :])


_NC_CACHE = None


def build():
    global _NC_CACHE
    if _NC_CACHE is None:
        nc = bacc.Bacc(
            "TRN2", target_bir_lowering=False, debug=False, num_devices=N_CORES
        )
        anc = nc.dram_tensor("anc", (BL, Z), F32, kind="ExternalInput").ap()
        pos = nc.dram_tensor("pos", (BL * NUM_POS, Z), F32, kind="ExternalInput").ap()
        neg = nc.dram_tensor("neg", (BL * NUM_NEG, Z), F32, kind="ExternalInput").ap()
        out = nc.dram_tensor("out", (P, NT), F32, kind="ExternalOutput").ap()
        with tile.TileContext(nc) as tc:
            _emit(tc, nc, anc, pos, neg, out)
        nc.compile()
        _NC_CACHE = nc
    return _NC_CACHE


def make_in_maps(anc_embedding, pos_embedding, neg_embedding):
    anc_embedding = np.asarray(anc_embedding, dtype=np.float32)
    pos_embedding = np.asarray(pos_embedding, dtype=np.float32)
    neg_embedding = np.asarray(neg_embedding, dtype=np.float32)
    in_maps = []
    for c in range(N_CORES):
        in_maps.append(
            {
                "anc": np.ascontiguousarray(anc_embedding[c * BL : (c + 1) * BL]),
                "pos": np.ascontiguousarray(
                    pos_embedding[c * BL * NUM_POS : (c + 1) * BL * NUM_POS]
                ),
                "neg": np.ascontiguousarray(
                    neg_embedding[c * BL * NUM_NEG : (c + 1) * BL * NUM_NEG]
                ),
            }
        )
    return in_maps


def combine(outs):
    # outs: list of [P, NT] per-core partial sums of relu((d_ap+m) - d_an)
    total = sum(o.astype(np.float64).sum() for o in outs)
    return np.float32(total / (B * NUM_POS * NUM_NEG))


def kernel(anc_embedding, pos_embedding, neg_embedding):
    nc = build()
    in_maps = make_in_maps(anc_embedding, pos_embedding, neg_embedding)
    res = bass_utils.run_bass_kernel_spmd(nc, in_maps, core_ids=list(range(N_CORES)))
    return combine([r["out"] for r in res.results])


# revision 6
# speedup vs baseline: 1.2751x; 1.2751x over previous
# Triplet-margin loss kernel for Trainium2 (Bass/Tile), batch-sharded
# across 8 NeuronCores.
#
# reference math (torch F.pairwise_distance semantics):
#   d_ap[b,p] = || anc[b] - pos[b,p] + eps ||_2
#   d_an[b,n] = || anc[b] - neg[b,n] + eps ||_2
#   loss = mean_{b,p,n} max(d_ap[b,p] - d_an[b,n] + margin, 0)
#
# (eps = 1e-6 shifts d^2 by ~3e-8 relative and is dropped; the whole
# pipeline computes in bf16 inputs + fp32 accumulation, which lands
# ~3e-5 relative on the final mean -- far under the 2e-2 gate.)
#
# Structure (pure DVE+ACT; GpSimd streaming shares an SBUF port with
# DVE and slows it ~1.5-8x, so GpSimd only *issues* DMAs, which is also
# the one engine whose DMAs can dtype-cast in flight):
#   x chunks and anc are DMA'd with an fp32 -> bf16 cast (HBM traffic
#   unchanged, SBUF halved, DVE/ACT 16-bit streaming is ~2x).
#   every slice j: dotm2[b,j] = -2 sum_z x*a  via one DVE STT (scalar
#   -2.0 folded into op0, fp32 accum) and nrm[b,j] = sum_z x^2 via one
#   ACT Square w/ accum, except K slices per tile whose norm runs on
#   DVE (self-multiply STT) to balance the two engines.
#   d = sqrt((nrm + dotm2) + ||a||^2)  -- one TT add + one biased Sqrt.
# The (p,n) pair combination is two ops on [128, P*N]: a broadcast
# (d_ap + margin) - d_an STT on DVE and a Relu w/ accum on ACT, giving
# the per-partition loss sum per tile. Each core returns [128, NT]
# partial sums; the host sums and scales.

import numpy as np

import concourse.bacc as bacc
import concourse.mybir as mybir
import concourse.tile as tile
from concourse import bass_utils

B, Z = 2048, 1024
NUM_POS, NUM_NEG = 8, 16
NJ = NUM_POS + NUM_NEG
MARGIN = 1.0
N_CORES = 8
BL = B // N_CORES  # 256 rows of anc per core
P = 128
NT = BL // P  # 2 batch-tiles per core

# chunk layout per tile: slice counts per DMA; small final chunks keep
# the post-DMA tail short.
CHUNK_SLICES = [4, 4, 4, 4, 4, 2, 2]
# slices whose norm is reduced on DVE instead of ACT (engine balance)
DVE_NRM = {5, 13, 21}

F32 = mybir.dt.float32
BF16 = mybir.dt.bfloat16
AF = mybir.ActivationFunctionType
OP = mybir.AluOpType


def _emit(tc, nc, anc, pos, neg, out):
    v = nc.vector
    act = nc.scalar
    gp = nc.gpsimd
    pos2 = pos.rearrange("(b j) z -> b (j z)", j=NUM_POS)  # [BL, 8*Z]
    neg2 = neg.rearrange("(b j) z -> b (j z)", j=NUM_NEG)  # [BL, 16*Z]

    starts = np.cumsum([0] + CHUNK_SLICES).tolist()
    assert starts[-1] == NJ

    def chunk_src(ci, b0):
        j0, j1 = starts[ci], starts[ci + 1]
        if j1 <= NUM_POS:
            return pos2[b0 : b0 + P, j0 * Z : j1 * Z]
        assert j0 >= NUM_POS
        return neg2[b0 : b0 + P, (j0 - NUM_POS) * Z : (j1 - NUM_POS) * Z]

    with (
        tc.tile_pool(name="xp", bufs=10) as xp,
        tc.tile_pool(name="apool", bufs=2) as apool,
        tc.tile_pool(name="scp", bufs=1) as scp,
        tc.tile_pool(name="smp", bufs=2) as smp,
        tc.tile_pool(name="opool", bufs=1) as opool,
    ):
        osb = opool.tile([P, NT], F32, name="osb")
        dve_scr = scp.tile([P, Z], BF16, name="dve_scr")
        act_scr = scp.tile([P, Z], BF16, name="act_scr")
        pair = scp.tile([P, NUM_POS * NUM_NEG], F32, name="pair")
        pair_scr = scp.tile([P, NUM_POS * NUM_NEG], F32, name="pair_scr")
        for t in range(NT):
            b0 = t * P
            anc_in = apool.tile([P, Z], BF16, name="anc_in")
            a_nrm = smp.tile([P, 1], F32, name="a_nrm")
            dotm2 = smp.tile([P, NJ], F32, name="dotm2")
            nrm = smp.tile([P, NJ], F32, name="nrm")
            d2c = smp.tile([P, NJ], F32, name="d2c")
            dt_ = smp.tile([P, NJ], F32, name="dt_")

            gp.dma_start(anc_in[:, :], anc[b0 : b0 + P, :])
            act.activation(
                act_scr[:, :], anc_in[:, :], AF.Square, accum_out=a_nrm[:, 0:1]
            )

            chunks = []
            for ci in range(len(CHUNK_SLICES)):
                xt = xp.tile([P, CHUNK_SLICES[ci] * Z], BF16, name="xt")
                gp.dma_start(xt[:, :], chunk_src(ci, b0))
                chunks.append(xt)

            def xs_of(jj):
                ci = next(i for i in range(len(starts) - 1) if starts[i + 1] > jj)
                q = jj - starts[ci]
                return chunks[ci][:, q * Z : (q + 1) * Z]

            # DVE: dotm2[:,jj] = sum((x * -2) * a); DVE-owned norms inline
            for jj in range(NJ):
                xs = xs_of(jj)
                v.scalar_tensor_tensor(
                    out=dve_scr[:, :],
                    in0=xs,
                    scalar=-2.0,
                    in1=anc_in[:, :],
                    op0=OP.mult,
                    op1=OP.mult,
                    accum_out=dotm2[:, jj : jj + 1],
                )
                if jj in DVE_NRM:
                    v.scalar_tensor_tensor(
                        out=dve_scr[:, :],
                        in0=xs,
                        scalar=1.0,
                        in1=xs,
                        op0=OP.bypass,
                        op1=OP.mult,
                        accum_out=nrm[:, jj : jj + 1],
                    )

            # ACT: nrm[:,jj] = sum x^2 for the rest
            for jj in range(NJ):
                if jj in DVE_NRM:
                    continue
                act.activation(
                    act_scr[:, :], xs_of(jj), AF.Square, accum_out=nrm[:, jj : jj + 1]
                )

            # d = sqrt((nrm + dotm2) + ||a||^2)
            v.tensor_tensor(out=d2c[:, :], in0=dotm2[:, :], in1=nrm[:, :], op=OP.add)
            act.activation(
                dt_[:, :], d2c[:, :], AF.Sqrt, bias=a_nrm[:, 0:1], scale=1.0
            )
            # pair[p,n] = (d_ap_p + margin) - d_an_n ; loss sum = sum relu
            v.scalar_tensor_tensor(
                out=pair[:, :].rearrange("p (a b) -> p a b", a=NUM_POS),
                in0=dt_[:, 0:NUM_POS, None].broadcast_to([P, NUM_POS, NUM_NEG]),
                scalar=MARGIN,
                in1=dt_[:, None, NUM_POS:NJ].broadcast_to([P, NUM_POS, NUM_NEG]),
                op0=OP.add,
                op1=OP.subtract,
            )
            act.activation(
                pair_scr[:, :], pair[:, :], AF.Relu, accum_out=osb[:, t : t + 1]
            )
        nc.sync.dma_start(out[:, :], osb[:, :])


_NC_CACHE = None


def build():
    global _NC_CACHE
    if _NC_CACHE is None:
        nc = bacc.Bacc(
            "TRN2", target_bir_lowering=False, debug=False, num_devices=N_CORES
        )
        anc = nc.dram_tensor("anc", (BL, Z), F32, kind="ExternalInput").ap()
        pos = nc.dram_tensor("pos", (BL * NUM_POS, Z), F32, kind="ExternalInput").ap()
        neg = nc.dram_tensor("neg", (BL * NUM_NEG, Z), F32, kind="ExternalInput").ap()
        out = nc.dram_tensor("out", (P, NT), F32, kind="ExternalOutput").ap()
        with tile.TileContext(nc) as tc:
            _emit(tc, nc, anc, pos, neg, out)
        nc.compile()
        _NC_CACHE = nc
    return _NC_CACHE


def make_in_maps(anc_embedding, pos_embedding, neg_embedding):
    anc_embedding = np.asarray(anc_embedding, dtype=np.float32)
    pos_embedding = np.asarray(pos_embedding, dtype=np.float32)
    neg_embedding = np.asarray(neg_embedding, dtype=np.float32)
    in_maps = []
    for c in range(N_CORES):
        in_maps.append(
            {
                "anc": np.ascontiguousarray(anc_embedding[c * BL : (c + 1) * BL]),
                "pos": np.ascontiguousarray(
                    pos_embedding[c * BL * NUM_POS : (c + 1) * BL * NUM_POS]
                ),
                "neg": np.ascontiguousarray(
                    neg_embedding[c * BL * NUM_NEG : (c + 1) * BL * NUM_NEG]
                ),
            }
        )
    return in_maps


def combine(outs):
    # outs: list of [P, NT] per-core partial sums of relu((d_ap+m) - d_an)
    total = sum(o.astype(np.float64).sum() for o in outs)
    return np.float32(total / (B * NUM_POS * NUM_NEG))


def kernel(anc_embedding, pos_embedding, neg_embedding):
    nc = build()
    in_maps = make_in_maps(anc_embedding, pos_embedding, neg_embedding)
    res = bass_utils.run_bass_kernel_spmd(nc, in_maps, core_ids=list(range(N_CORES)))
    return combine([r["out"] for r in res.results])


# revision 7
# speedup vs baseline: 1.3979x; 1.0964x over previous
# Triplet-margin loss kernel for Trainium2 (Bass/Tile), batch-sharded
# across 8 NeuronCores.
#
# reference math (torch F.pairwise_distance semantics):
#   d_ap[b,p] = || anc[b] - pos[b,p] + eps ||_2
#   d_an[b,n] = || anc[b] - neg[b,n] + eps ||_2
#   loss = mean_{b,p,n} max(d_ap[b,p] - d_an[b,n] + margin, 0)
#
# (eps = 1e-6 shifts d^2 by ~3e-8 relative and is dropped; the whole
# pipeline computes on bf16 inputs with fp32 accumulation, which lands
# ~1e-4 relative on the final mean -- far under the 2e-2 gate.)
#
# Engine strategy. DVE reductions (accum ops) run 1 elem/cycle in every
# mode, but plain TensorTensor in bf16 runs 2x_1p (2 elem/cycle). So:
#   - x chunks and anc are DMA'd with an fp32 -> bf16 cast in flight
#     (GpSimd-issued DMAs are the one kind that can cast; HBM traffic
#     is unchanged, SBUF halves, and bf16 enables the 2x subtract).
#   - u = x - anc: one wide bf16 TT per chunk on DVE at 2x (the anc
#     operand rides a stride-0 broadcast AP).
#   - d^2 = sum u^2: ONE 1x reduction per slice -- DVE self-multiply
#     STT w/ fp32 accum or ACT Square w/ accum, split K:(24-K) per tile
#     to balance the engines. (The dot+norm decomposition needs TWO 1x
#     reductions per slice; this needs one plus a half-cost subtract.)
#   - d = sqrt(d^2): one unbiased ACT Sqrt per tile.
#   - pair loss: broadcast (d_ap + margin) - d_an STT on DVE, then
#     Relu w/ accum on ACT -> per-partition loss sums, [128, NT] out.
# GpSimd never streams (its SBUF port is shared with DVE and concurrent
# streaming slows DVE 1.5-8x); it only triggers the cast DMAs.

import numpy as np

import concourse.bacc as bacc
import concourse.mybir as mybir
import concourse.tile as tile
from concourse import bass_utils

B, Z = 2048, 1024
NUM_POS, NUM_NEG = 8, 16
NJ = NUM_POS + NUM_NEG
MARGIN = 1.0
N_CORES = 8
BL = B // N_CORES  # 256 rows of anc per core
P = 128
NT = BL // P  # 2 batch-tiles per core

# chunk layout per tile: slice counts per DMA; small final chunks keep
# the post-DMA tail short.
CHUNK_SLICES = [4, 4, 4, 4, 4, 2, 2]
# slices whose d^2 reduction runs on DVE instead of ACT (engine balance)
DVE_NRM = {0, 3, 6, 9, 12, 15, 18, 21}

F32 = mybir.dt.float32
BF16 = mybir.dt.bfloat16
AF = mybir.ActivationFunctionType
OP = mybir.AluOpType


def _emit(tc, nc, anc, pos, neg, out):
    v = nc.vector
    act = nc.scalar
    gp = nc.gpsimd
    pos2 = pos.rearrange("(b j) z -> b (j z)", j=NUM_POS)  # [BL, 8*Z]
    neg2 = neg.rearrange("(b j) z -> b (j z)", j=NUM_NEG)  # [BL, 16*Z]

    starts = np.cumsum([0] + CHUNK_SLICES).tolist()
    assert starts[-1] == NJ

    def chunk_src(ci, b0):
        j0, j1 = starts[ci], starts[ci + 1]
        if j1 <= NUM_POS:
            return pos2[b0 : b0 + P, j0 * Z : j1 * Z]
        assert j0 >= NUM_POS
        return neg2[b0 : b0 + P, (j0 - NUM_POS) * Z : (j1 - NUM_POS) * Z]

    with (
        tc.tile_pool(name="xp", bufs=10) as xp,
        tc.tile_pool(name="up", bufs=5) as up,
        tc.tile_pool(name="apool", bufs=2) as apool,
        tc.tile_pool(name="scp", bufs=1) as scp,
        tc.tile_pool(name="smp", bufs=2) as smp,
        tc.tile_pool(name="opool", bufs=1) as opool,
    ):
        osb = opool.tile([P, NT], F32, name="osb")
        dve_scr = scp.tile([P, Z], BF16, name="dve_scr")
        act_scr = scp.tile([P, Z], BF16, name="act_scr")
        pair = scp.tile([P, NUM_POS * NUM_NEG], F32, name="pair")
        pair_scr = scp.tile([P, NUM_POS * NUM_NEG], F32, name="pair_scr")
        for t in range(NT):
            b0 = t * P
            anc_in = apool.tile([P, Z], BF16, name="anc_in")
            nrm = smp.tile([P, NJ], F32, name="nrm")
            dt_ = smp.tile([P, NJ], F32, name="dt_")

            gp.dma_start(anc_in[:, :], anc[b0 : b0 + P, :])

            chunks = []
            for ci in range(len(CHUNK_SLICES)):
                xt = xp.tile([P, CHUNK_SLICES[ci] * Z], BF16, name="xt")
                gp.dma_start(xt[:, :], chunk_src(ci, b0))
                chunks.append(xt)

            # DVE: u = x - anc, one wide 2x bf16 TT per chunk
            us = {}
            for ci, cw in enumerate(CHUNK_SLICES):
                ut = up.tile([P, cw * Z], BF16, name="ut")
                v.tensor_tensor(
                    out=ut[:, :].rearrange("p (c z) -> p c z", c=cw),
                    in0=chunks[ci][:, :].rearrange("p (c z) -> p c z", c=cw),
                    in1=anc_in[:, None, :].broadcast_to([P, cw, Z]),
                    op=OP.subtract,
                )
                for q in range(cw):
                    us[starts[ci] + q] = ut[:, q * Z : (q + 1) * Z]

                # d^2 reductions for this chunk's slices, interleaved so
                # both engines start as soon as each chunk's u exists
                for q in range(cw):
                    jj = starts[ci] + q
                    if jj in DVE_NRM:
                        v.scalar_tensor_tensor(
                            out=dve_scr[:, :],
                            in0=us[jj],
                            scalar=1.0,
                            in1=us[jj],
                            op0=OP.bypass,
                            op1=OP.mult,
                            accum_out=nrm[:, jj : jj + 1],
                        )
                    else:
                        act.activation(
                            act_scr[:, :],
                            us[jj],
                            AF.Square,
                            accum_out=nrm[:, jj : jj + 1],
                        )

            # d = sqrt(d^2)
            act.activation(dt_[:, :], nrm[:, :], AF.Sqrt)
            # pair[p,n] = (d_ap_p + margin) - d_an_n ; loss sum = sum relu
            v.scalar_tensor_tensor(
                out=pair[:, :].rearrange("p (a b) -> p a b", a=NUM_POS),
                in0=dt_[:, 0:NUM_POS, None].broadcast_to([P, NUM_POS, NUM_NEG]),
                scalar=MARGIN,
                in1=dt_[:, None, NUM_POS:NJ].broadcast_to([P, NUM_POS, NUM_NEG]),
                op0=OP.add,
                op1=OP.subtract,
            )
            act.activation(
                pair_scr[:, :], pair[:, :], AF.Relu, accum_out=osb[:, t : t + 1]
            )
        nc.sync.dma_start(out[:, :], osb[:, :])


_NC_CACHE = None


def build():
    global _NC_CACHE
    if _NC_CACHE is None:
        nc = bacc.Bacc(
            "TRN2", target_bir_lowering=False, debug=False, num_devices=N_CORES
        )
        anc = nc.dram_tensor("anc", (BL, Z), F32, kind="ExternalInput").ap()
        pos = nc.dram_tensor("pos", (BL * NUM_POS, Z), F32, kind="ExternalInput").ap()
        neg = nc.dram_tensor("neg", (BL * NUM_NEG, Z), F32, kind="ExternalInput").ap()
        out = nc.dram_tensor("out", (P, NT), F32, kind="ExternalOutput").ap()
        with tile.TileContext(nc) as tc:
            _emit(tc, nc, anc, pos, neg, out)
        nc.compile()
        _NC_CACHE = nc
    return _NC_CACHE


def make_in_maps(anc_embedding, pos_embedding, neg_embedding):
    anc_embedding = np.asarray(anc_embedding, dtype=np.float32)
    pos_embedding = np.asarray(pos_embedding, dtype=np.float32)
    neg_embedding = np.asarray(neg_embedding, dtype=np.float32)
    in_maps = []
    for c in range(N_CORES):
        in_maps.append(
            {
                "anc": np.ascontiguousarray(anc_embedding[c * BL : (c + 1) * BL]),
                "pos": np.ascontiguousarray(
                    pos_embedding[c * BL * NUM_POS : (c + 1) * BL * NUM_POS]
                ),
                "neg": np.ascontiguousarray(
                    neg_embedding[c * BL * NUM_NEG : (c + 1) * BL * NUM_NEG]
                ),
            }
        )
    return in_maps


def combine(outs):
    # outs: list of [P, NT] per-core partial sums of relu((d_ap+m) - d_an)
    total = sum(o.astype(np.float64).sum() for o in outs)
    return np.float32(total / (B * NUM_POS * NUM_NEG))


def kernel(anc_embedding, pos_embedding, neg_embedding):
    nc = build()
    in_maps = make_in_maps(anc_embedding, pos_embedding, neg_embedding)
    res = bass_utils.run_bass_kernel_spmd(nc, in_maps, core_ids=list(range(N_CORES)))
    return combine([r["out"] for r in res.results])


# revision 10
# speedup vs baseline: 1.4350x; 1.0265x over previous
# Triplet-margin loss kernel for Trainium2 (Bass/Tile), batch-sharded
# across 8 NeuronCores.
#
# reference math (torch F.pairwise_distance semantics):
#   d_ap[b,p] = || anc[b] - pos[b,p] + eps ||_2
#   d_an[b,n] = || anc[b] - neg[b,n] + eps ||_2
#   loss = mean_{b,p,n} max(d_ap[b,p] - d_an[b,n] + margin, 0)
#
# (eps = 1e-6 shifts d^2 by ~3e-8 relative and is dropped; the whole
# pipeline computes on bf16 inputs with fp32 accumulation, which lands
# ~1e-4 relative on the final mean -- far under the 2e-2 gate.)
#
# Engine strategy. DVE reductions (accum ops) run 1 elem/cycle in every
# mode, but plain TensorTensor in bf16 runs 2x_1p (2 elem/cycle). So:
#   - x chunks and anc are DMA'd with an fp32 -> bf16 cast in flight
#     (GpSimd-issued DMAs are the one kind that can cast; HBM traffic
#     is unchanged, SBUF halves, and bf16 enables the 2x subtract).
#   - u = x - anc: one wide bf16 TT per chunk on DVE at 2x (the anc
#     operand rides a stride-0 broadcast AP).
#   - d^2 = sum u^2: ONE 1x reduction per slice -- DVE self-multiply
#     STT w/ fp32 accum or ACT Square w/ accum, split K:(24-K) per tile
#     to balance the engines. (The dot+norm decomposition needs TWO 1x
#     reductions per slice; this needs one plus a half-cost subtract.)
#   - d = sqrt(d^2): one unbiased ACT Sqrt per tile.
#   - pair loss: broadcast (d_ap + margin) - d_an STT on DVE, then
#     Relu w/ accum on ACT -> per-partition loss sums, [128, NT] out.
# GpSimd never streams (its SBUF port is shared with DVE and concurrent
# streaming slows DVE 1.5-8x); it only triggers the cast DMAs.

import numpy as np

import concourse.bacc as bacc
import concourse.mybir as mybir
import concourse.tile as tile
from concourse import bass_utils

B, Z = 2048, 1024
NUM_POS, NUM_NEG = 8, 16
NJ = NUM_POS + NUM_NEG
MARGIN = 1.0
N_CORES = 8
BL = B // N_CORES  # 256 rows of anc per core
P = 128
NT = BL // P  # 2 batch-tiles per core

# chunk layout per tile: slice counts per DMA; small final chunks keep
# the post-DMA tail short.
CHUNK_SLICES = [4, 4, 4, 4, 4, 2, 2]
# slices whose d^2 reduction runs on DVE instead of ACT (engine balance)
DVE_NRM = {0, 3, 6, 9, 12, 15, 18, 21}

F32 = mybir.dt.float32
BF16 = mybir.dt.bfloat16
AF = mybir.ActivationFunctionType
OP = mybir.AluOpType


def _emit(tc, nc, anc, pos, neg, out):
    v = nc.vector
    act = nc.scalar
    gp = nc.gpsimd
    pos2 = pos.rearrange("(b j) z -> b (j z)", j=NUM_POS)  # [BL, 8*Z]
    neg2 = neg.rearrange("(b j) z -> b (j z)", j=NUM_NEG)  # [BL, 16*Z]

    starts = np.cumsum([0] + CHUNK_SLICES).tolist()
    assert starts[-1] == NJ

    def chunk_src(ci, b0):
        j0, j1 = starts[ci], starts[ci + 1]
        if j1 <= NUM_POS:
            return pos2[b0 : b0 + P, j0 * Z : j1 * Z]
        assert j0 >= NUM_POS
        return neg2[b0 : b0 + P, (j0 - NUM_POS) * Z : (j1 - NUM_POS) * Z]

    with (
        tc.tile_pool(name="xp", bufs=10) as xp,
        tc.tile_pool(name="up", bufs=5) as up,
        tc.tile_pool(name="apool", bufs=2) as apool,
        tc.tile_pool(name="scp", bufs=1) as scp,
        tc.tile_pool(name="smp", bufs=2) as smp,
        tc.tile_pool(name="opool", bufs=1) as opool,
    ):
        osb = opool.tile([P, 2 * NT], F32, name="osb")
        dve_scr = scp.tile([P, Z], BF16, name="dve_scr")
        act_scr = scp.tile([P, Z], BF16, name="act_scr")
        pair = scp.tile([P, NUM_POS * NUM_NEG], F32, name="pair")
        pair_scr = scp.tile([P, NUM_POS * NUM_NEG], F32, name="pair_scr")
        for t in range(NT):
            b0 = t * P
            anc_in = apool.tile([P, Z], BF16, name="anc_in")
            nrm = smp.tile([P, NJ], F32, name="nrm")
            dt_ = smp.tile([P, NJ], F32, name="dt_")

            gp.dma_start(anc_in[:, :], anc[b0 : b0 + P, :])

            chunks = []
            for ci in range(len(CHUNK_SLICES)):
                xt = xp.tile([P, CHUNK_SLICES[ci] * Z], BF16, name="xt")
                gp.dma_start(xt[:, :], chunk_src(ci, b0))
                chunks.append(xt)

            # DVE: u = x - anc, one wide 2x bf16 TT per chunk
            us = {}
            for ci, cw in enumerate(CHUNK_SLICES):
                ut = up.tile([P, cw * Z], BF16, name="ut")
                v.tensor_tensor(
                    out=ut[:, :].rearrange("p (c z) -> p c z", c=cw),
                    in0=chunks[ci][:, :].rearrange("p (c z) -> p c z", c=cw),
                    in1=anc_in[:, None, :].broadcast_to([P, cw, Z]),
                    op=OP.subtract,
                )
                for q in range(cw):
                    us[starts[ci] + q] = ut[:, q * Z : (q + 1) * Z]

                # d^2 reductions for this chunk's slices, interleaved so
                # both engines start as soon as each chunk's u exists
                for q in range(cw):
                    jj = starts[ci] + q
                    if jj in DVE_NRM:
                        v.scalar_tensor_tensor(
                            out=dve_scr[:, :],
                            in0=us[jj],
                            scalar=1.0,
                            in1=us[jj],
                            op0=OP.bypass,
                            op1=OP.mult,
                            accum_out=nrm[:, jj : jj + 1],
                        )
                    else:
                        act.activation(
                            act_scr[:, :],
                            us[jj],
                            AF.Square,
                            accum_out=nrm[:, jj : jj + 1],
                        )

            # d = sqrt(d^2), split so only the last chunk's columns gate
            # the post-DMA tail; pair[p,n] = (d_ap_p + margin) - d_an_n
            # and the loss sum = sum relu(pair), likewise split by n.
            LATE = CHUNK_SLICES[-1]  # d_an columns in the final chunk
            NA = NUM_NEG - LATE
            act.activation(dt_[:, 0 : NJ - LATE], nrm[:, 0 : NJ - LATE], AF.Sqrt)
            v.scalar_tensor_tensor(
                out=pair[:, 0 : NUM_POS * NA].rearrange("p (a b) -> p a b", a=NUM_POS),
                in0=dt_[:, 0:NUM_POS, None].broadcast_to([P, NUM_POS, NA]),
                scalar=MARGIN,
                in1=dt_[:, None, NUM_POS : NJ - LATE].broadcast_to([P, NUM_POS, NA]),
                op0=OP.add,
                op1=OP.subtract,
            )
            act.activation(
                pair_scr[:, 0 : NUM_POS * NA],
                pair[:, 0 : NUM_POS * NA],
                AF.Relu,
                accum_out=osb[:, 2 * t : 2 * t + 1],
            )
            act.activation(dt_[:, NJ - LATE : NJ], nrm[:, NJ - LATE : NJ], AF.Sqrt)
            v.scalar_tensor_tensor(
                out=pair[:, NUM_POS * NA : NUM_POS * NUM_NEG].rearrange(
                    "p (a b) -> p a b", a=NUM_POS
                ),
                in0=dt_[:, 0:NUM_POS, None].broadcast_to([P, NUM_POS, LATE]),
                scalar=MARGIN,
                in1=dt_[:, None, NJ - LATE : NJ].broadcast_to([P, NUM_POS, LATE]),
                op0=OP.add,
                op1=OP.subtract,
            )
            act.activation(
                pair_scr[:, NUM_POS * NA : NUM_POS * NUM_NEG],
                pair[:, NUM_POS * NA : NUM_POS * NUM_NEG],
                AF.Relu,
                accum_out=osb[:, 2 * t + 1 : 2 * t + 2],
            )
        nc.sync.dma_start(out[:, :], osb[:, :])


_NC_CACHE = None


def build():
    global _NC_CACHE
    if _NC_CACHE is None:
        nc = bacc.Bacc(
            "TRN2", target_bir_lowering=False, debug=False, num_devices=N_CORES
        )
        anc = nc.dram_tensor("anc", (BL, Z), F32, kind="ExternalInput").ap()
        pos = nc.dram_tensor("pos", (BL * NUM_POS, Z), F32, kind="ExternalInput").ap()
        neg = nc.dram_tensor("neg", (BL * NUM_NEG, Z), F32, kind="ExternalInput").ap()
        out = nc.dram_tensor("out", (P, 2 * NT), F32, kind="ExternalOutput").ap()
        with tile.TileContext(nc) as tc:
            _emit(tc, nc, anc, pos, neg, out)
        nc.compile()
        _NC_CACHE = nc
    return _NC_CACHE


def make_in_maps(anc_embedding, pos_embedding, neg_embedding):
    anc_embedding = np.asarray(anc_embedding, dtype=np.float32)
    pos_embedding = np.asarray(pos_embedding, dtype=np.float32)
    neg_embedding = np.asarray(neg_embedding, dtype=np.float32)
    in_maps = []
    for c in range(N_CORES):
        in_maps.append(
            {
                "anc": np.ascontiguousarray(anc_embedding[c * BL : (c + 1) * BL]),
                "pos": np.ascontiguousarray(
                    pos_embedding[c * BL * NUM_POS : (c + 1) * BL * NUM_POS]
                ),
                "neg": np.ascontiguousarray(
                    neg_embedding[c * BL * NUM_NEG : (c + 1) * BL * NUM_NEG]
                ),
            }
        )
    return in_maps


def combine(outs):
    # outs: list of [P, NT] per-core partial sums of relu((d_ap+m) - d_an)
    total = sum(o.astype(np.float64).sum() for o in outs)
    return np.float32(total / (B * NUM_POS * NUM_NEG))


def kernel(anc_embedding, pos_embedding, neg_embedding):
    nc = build()
    in_maps = make_in_maps(anc_embedding, pos_embedding, neg_embedding)
    res = bass_utils.run_bass_kernel_spmd(nc, in_maps, core_ids=list(range(N_CORES)))
    return combine([r["out"] for r in res.results])
